# revision 2
# baseline (speedup 1.0000x reference)
"""AttentionBlock (GroupNorm32 + QKV 8-head attention + proj + residual) on 8 TRN2 NeuronCores.

Sharding: pure data-parallel over batch B=8 — one batch element per core.
Per core: x [512, 1024] f32 -> GroupNorm(32) -> qkv (bf16 matmuls) ->
8-head attention (softmax without max-subtraction; logits O(1)) -> proj + residual.

Layout tricks:
  - All big matmuls in bf16 (residual path stays exact f32, so output error ~1e-3).
  - Attention weights computed TRANSPOSED: wT[s, t] = k.T @ q, so the AV matmul needs
    no transposes; V is produced transposed by swapping the qkv matmul operands.
  - Softmax row-sums come free via a ones-column appended to the stationary vT operand.
  - Softmax division is a per-t (free axis) scale: reciprocal row -> PE broadcast
    matmul (ones lhsT) -> fused into the PSUM evacuation multiply.
"""

import numpy as np
import ml_dtypes
from contextlib import ExitStack

import concourse.bass as bass
import concourse.tile as tile
from concourse import bacc, mybir
from concourse.bass_utils import run_bass_kernel_spmd

F32 = mybir.dt.float32
BF = mybir.dt.bfloat16
MULT = mybir.AluOpType.mult
ADD = mybir.AluOpType.add
SUB = mybir.AluOpType.subtract
AFT = mybir.ActivationFunctionType

C, T, H, CH = 512, 1024, 8, 64
NJ = C // 128          # 4 c-tiles
NTM = T // 128         # 8 t-tiles
EPS = 1e-5
EXP_SCALE = float(CH) ** -0.5  # folded (q*s)·(k*s) scale, s = ch**-0.25

BF_NP = ml_dtypes.bfloat16


def build_graph(enable_asserts: bool = False):
    nc = bacc.Bacc(
        "TRN2",
        target_bir_lowering=False,
        debug=False,
        enable_asserts=enable_asserts,
    )
    x_d = nc.dram_tensor("x", [C, T], F32, kind="ExternalInput").ap()
    wq_d = nc.dram_tensor("wq", [C, C], BF, kind="ExternalInput").ap()
    wk_d = nc.dram_tensor("wk", [C, C], BF, kind="ExternalInput").ap()
    wv_d = nc.dram_tensor("wv", [C, C], BF, kind="ExternalInput").ap()
    pw_d = nc.dram_tensor("pw", [C, C], BF, kind="ExternalInput").ap()
    bq_d = nc.dram_tensor("bq", [C], F32, kind="ExternalInput").ap()
    bk_d = nc.dram_tensor("bk", [C], F32, kind="ExternalInput").ap()
    bv_d = nc.dram_tensor("bv", [C], F32, kind="ExternalInput").ap()
    pb_d = nc.dram_tensor("pb", [C], F32, kind="ExternalInput").ap()
    gns_d = nc.dram_tensor("gns", [C], F32, kind="ExternalInput").ap()
    gnb_d = nc.dram_tensor("gnb", [C], F32, kind="ExternalInput").ap()
    g8_d = nc.dram_tensor("g8", [128, 8], F32, kind="ExternalInput").ap()
    gt8_d = nc.dram_tensor("gt8", [8, 128], F32, kind="ExternalInput").ap()
    out_d = nc.dram_tensor("out", [C, T], F32, kind="ExternalOutput").ap()

    with tile.TileContext(nc) as tc, ExitStack() as ctx:
        consts = ctx.enter_context(tc.tile_pool(name="consts", bufs=1))
        bigs = ctx.enter_context(tc.tile_pool(name="bigs", bufs=1))
        ewp = ctx.enter_context(tc.tile_pool(name="ewp", bufs=2))
        work = ctx.enter_context(tc.tile_pool(name="work", bufs=3))
        outp = ctx.enter_context(tc.tile_pool(name="outp", bufs=2))
        qk_ps = ctx.enter_context(tc.tile_pool(name="qk_ps", bufs=2, space="PSUM"))
        av_ps = ctx.enter_context(tc.tile_pool(name="av_ps", bufs=2, space="PSUM"))
        bc_ps = ctx.enter_context(tc.tile_pool(name="bc_ps", bufs=1, space="PSUM"))

        # ---- persistent sbuf tensors ----
        xt = bigs.tile([128, NJ, T], F32)       # raw x, kept for residual
        xn = bigs.tile([128, NJ, T], BF)        # groupnormed x
        q_sb = bigs.tile([128, NJ, T], BF)      # q rows (head-major)
        k_sb = bigs.tile([128, NJ, T], BF)      # k rows (head-major)
        vT_sb = bigs.tile([128, NTM, H, CH + 1], BF)  # v transposed + ones col
        a_sb = bigs.tile([128, NJ, T], BF)      # normalized attention output

        # ---- input DMAs (ordered by first use) ----
        for j in range(NJ):
            nc.sync.dma_start(xt[:, j, :], x_d[j * 128:(j + 1) * 128, :])
        gns_sb = consts.tile([128, NJ], F32)
        gnb_sb = consts.tile([128, NJ], F32)
        nc.sync.dma_start(gns_sb[:], bass.AP(tensor=gns_d.tensor, offset=0, ap=[[1, 128], [128, NJ]]))
        nc.sync.dma_start(gnb_sb[:], bass.AP(tensor=gnb_d.tensor, offset=0, ap=[[1, 128], [128, NJ]]))
        g8_sb = consts.tile([128, 8], F32)
        gt8_sb = consts.tile([8, 128], F32)
        nc.sync.dma_start(g8_sb[:], g8_d[:])
        nc.sync.dma_start(gt8_sb[:], gt8_d[:])

        wq_sb = consts.tile([128, NJ, C], BF)
        wk_sb = consts.tile([128, NJ, C], BF)
        wv_sb = consts.tile([128, NJ, C], BF)
        pw_sb = consts.tile([128, NJ, C], BF)
        for j in range(NJ):
            nc.sync.dma_start(wk_sb[:, j, :], wk_d[j * 128:(j + 1) * 128, :])
            nc.sync.dma_start(wq_sb[:, j, :], wq_d[j * 128:(j + 1) * 128, :])
            nc.sync.dma_start(wv_sb[:, j, :], wv_d[j * 128:(j + 1) * 128, :])
            nc.sync.dma_start(pw_sb[:, j, :], pw_d[j * 128:(j + 1) * 128, :])
        bq_sb = consts.tile([128, NJ], F32)
        bk_sb = consts.tile([128, NJ], F32)
        pb_sb = consts.tile([128, NJ], F32)
        for j in range(NJ):
            nc.sync.dma_start(bq_sb[:, j:j + 1], bq_d[j * 128:(j + 1) * 128])
            nc.sync.dma_start(bk_sb[:, j:j + 1], bk_d[j * 128:(j + 1) * 128])
            nc.sync.dma_start(pb_sb[:, j:j + 1], pb_d[j * 128:(j + 1) * 128])
        bv_bc = consts.tile([128, C], F32)      # v bias broadcast to all partitions
        nc.sync.dma_start(bv_bc[:], bass.AP(tensor=bv_d.tensor, offset=0, ap=[[0, 128], [1, C]]))
        ones_row = consts.tile([1, 64], BF)
        nc.vector.memset(ones_row[:], 1.0)
        eps_sb = consts.tile([128, 1], F32)
        nc.vector.memset(eps_sb[:], EPS)
        zero_sb = consts.tile([128, 1], F32)
        nc.vector.memset(zero_sb[:], 0.0)

        # ---- GroupNorm: per-partition stats, group-reduce via tiny f32 matmuls ----
        stats_sb = consts.tile([128, 3 * NJ], F32)  # mean | var | mean^2 per c-tile
        for j in range(NJ):
            st6 = work.tile([128, 2, 6], F32, tag="st6")
            nc.vector.bn_stats(st6[:, 0, :], xt[:, j, 0:512])
            nc.vector.bn_stats(st6[:, 1, :], xt[:, j, 512:1024])
            nc.vector.bn_aggr(stats_sb[:, 3 * j:3 * j + 2], st6[:])
            nc.vector.tensor_mul(stats_sb[:, 3 * j + 2:3 * j + 3],
                                 stats_sb[:, 3 * j:3 * j + 1],
                                 stats_sb[:, 3 * j:3 * j + 1])
        ps_st = av_ps.tile([8, 3 * NJ], F32, tag="av")
        nc.tensor.matmul(ps_st[:], g8_sb[:], stats_sb[:], start=True, stop=True)
        st_g = work.tile([8, 3 * NJ], F32, tag="stg")
        nc.vector.tensor_scalar(st_g[:], ps_st[:], 1.0 / 16.0, None, op0=MULT)
        stv = st_g[:].rearrange("p (j c) -> p j c", c=3)
        bcin = work.tile([8, 8], F32, tag="bcin")
        vv = work.tile([8, NJ], F32, tag="vv")
        nc.vector.tensor_add(vv[:], stv[:, :, 1], stv[:, :, 2])
        m2 = work.tile([8, NJ], F32, tag="m2")
        nc.vector.tensor_mul(m2[:], stv[:, :, 0], stv[:, :, 0])
        nc.vector.tensor_sub(vv[:], vv[:], m2[:])
        nc.scalar.activation(vv[:], vv[:], AFT.Sqrt, bias=eps_sb[0:8, :], scale=1.0)
        nc.vector.tensor_copy(bcin[:, 0:4], stv[:, :, 0])
        nc.vector.reciprocal(bcin[:, 4:8], vv[:])
        ps_pp = av_ps.tile([128, 8], F32, tag="av")
        nc.tensor.matmul(ps_pp[:], gt8_sb[:], bcin[:], start=True, stop=True)
        ab = work.tile([128, 2 * NJ], F32, tag="ab")   # scale | shift per c-tile
        t1 = work.tile([128, 1], F32, tag="t1")
        for j in range(NJ):
            nc.vector.tensor_mul(ab[:, j:j + 1], ps_pp[:, 4 + j:5 + j], gns_sb[:, j:j + 1])
            nc.vector.tensor_mul(t1[:], ps_pp[:, j:j + 1], ab[:, j:j + 1])
            nc.vector.tensor_sub(ab[:, 4 + j:5 + j], gnb_sb[:, j:j + 1], t1[:])
        for j in range(NJ):
            nc.vector.tensor_scalar(xn[:, j, :], xt[:, j, :],
                                    ab[:, j:j + 1], ab[:, 4 + j:5 + j],
                                    op0=MULT, op1=ADD)

        # ---- QKV ----
        # k, q: out[o, t] = Wk/Wq^T(lhsT [c,o]) x xn[c, t]
        for m in range(NJ):
            psk = qk_ps.tile([128, T], F32, tag="qk")
            for n in range(2):
                for j in range(NJ):
                    nc.tensor.matmul(psk[:, 512 * n:512 * (n + 1)],
                                     wk_sb[:, j, 128 * m:128 * (m + 1)],
                                     xn[:, j, 512 * n:512 * (n + 1)],
                                     start=(j == 0), stop=(j == NJ - 1))
            nc.vector.tensor_scalar(k_sb[:, m, :], psk[:], bk_sb[:, m:m + 1], None, op0=ADD)
            psq = qk_ps.tile([128, T], F32, tag="qk")
            for n in range(2):
                for j in range(NJ):
                    nc.tensor.matmul(psq[:, 512 * n:512 * (n + 1)],
                                     wq_sb[:, j, 128 * m:128 * (m + 1)],
                                     xn[:, j, 512 * n:512 * (n + 1)],
                                     start=(j == 0), stop=(j == NJ - 1))
            nc.vector.tensor_scalar(q_sb[:, m, :], psq[:], bq_sb[:, m:m + 1], None, op0=ADD)
        # vT: out[t, ov] = xn(lhsT [c,t]) x Wv([c, ov])
        for tm in range(NTM):
            psv = qk_ps.tile([128, T], F32, tag="qk")
            for j in range(NJ):
                nc.tensor.matmul(psv[:, 0:512],
                                 xn[:, j, 128 * tm:128 * (tm + 1)],
                                 wv_sb[:, j, :],
                                 start=(j == 0), stop=(j == NJ - 1))
            nc.vector.tensor_add(vT_sb[:, tm, :, 0:CH],
                                 psv[:, 0:512].rearrange("p (h c) -> p h c", h=H),
                                 bv_bc[:].rearrange("p (h c) -> p h c", h=H))
            nc.vector.memset(vT_sb[:, tm, :, CH:CH + 1], 1.0)

        # ---- attention, head pairs (2p at partitions 0:64, 2p+1 at 64:128) ----
        for p in range(NJ):
            ew = ewp.tile([128, NTM, 2, T], BF, tag="ew")
            for sm in range(NTM):
                for n in range(2):
                    psw = qk_ps.tile([128, T], F32, tag="qk")
                    nc.tensor.matmul(psw[:, 0:512],
                                     k_sb[0:64, p, 128 * sm:128 * (sm + 1)],
                                     q_sb[0:64, p, 512 * n:512 * (n + 1)],
                                     start=True, stop=True, tile_position=(0, 0))
                    nc.tensor.matmul(psw[:, 512:1024],
                                     k_sb[64:128, p, 128 * sm:128 * (sm + 1)],
                                     q_sb[64:128, p, 512 * n:512 * (n + 1)],
                                     start=True, stop=True, tile_position=(64, 0))
                    nc.scalar.activation(ew[:, sm, :, 512 * n:512 * (n + 1)],
                                         psw[:].rearrange("p (u t) -> p u t", u=2),
                                         AFT.Exp, bias=zero_sb[:], scale=EXP_SCALE)
            for u in range(2):
                h = 2 * p + u
                for n in range(2):
                    psa = av_ps.tile([CH + 1, 512], F32, tag="av")
                    for sm in range(NTM):
                        nc.tensor.matmul(psa[:],
                                         vT_sb[:, sm, h, :],
                                         ew[:, sm, u, 512 * n:512 * (n + 1)],
                                         start=(sm == 0), stop=(sm == NTM - 1))
                    rec = work.tile([1, 512], BF, tag="rec")
                    with nc.allow_low_precision(reason="softmax denom bf16"):
                        nc.vector.reciprocal(rec[:], psa[CH:CH + 1, :])
                    psb = bc_ps.tile([64, 512], F32, tag="bc")
                    nc.tensor.matmul(psb[:], ones_row[:], rec[:], start=True, stop=True)
                    bcs = work.tile([64, 512], BF, tag="bcs")
                    nc.vector.tensor_copy(bcs[:], psb[:])
                    nc.vector.tensor_mul(a_sb[64 * u:64 * (u + 1), p, 512 * n:512 * (n + 1)],
                                         psa[0:CH, :], bcs[:])

        # ---- proj + residual ----
        for m in range(NJ):
            psp = qk_ps.tile([128, T], F32, tag="qk")
            for n in range(2):
                for j in range(NJ):
                    nc.tensor.matmul(psp[:, 512 * n:512 * (n + 1)],
                                     pw_sb[:, j, 128 * m:128 * (m + 1)],
                                     a_sb[:, j, 512 * n:512 * (n + 1)],
                                     start=(j == 0), stop=(j == NJ - 1))
            osb = outp.tile([128, T], F32, tag="osb")
            nc.vector.scalar_tensor_tensor(osb[:], psp[:], pb_sb[:, m:m + 1], xt[:, m, :],
                                           op0=ADD, op1=ADD)
            nc.sync.dma_start(out_d[128 * m:128 * (m + 1), :], osb[:])

    nc.compile()
    return nc


_NC_CACHE = {}


def get_nc():
    if "nc" not in _NC_CACHE:
        _NC_CACHE["nc"] = build_graph()
    return _NC_CACHE["nc"]


def make_in_maps(x, norm_scale, norm_bias, qkv_w, qkv_b, proj_w, proj_b):
    x = np.asarray(x, dtype=np.float32)
    B = x.shape[0]
    qr = np.asarray(qkv_w, np.float32).reshape(H, 3, CH, C)
    wq = np.ascontiguousarray(qr[:, 0].reshape(C, C).T).astype(BF_NP)
    wk = np.ascontiguousarray(qr[:, 1].reshape(C, C).T).astype(BF_NP)
    wv = np.ascontiguousarray(qr[:, 2].reshape(C, C).T).astype(BF_NP)
    br = np.asarray(qkv_b, np.float32).reshape(H, 3, CH)
    bq = np.ascontiguousarray(br[:, 0].reshape(C))
    bk = np.ascontiguousarray(br[:, 1].reshape(C))
    bv = np.ascontiguousarray(br[:, 2].reshape(C))
    pw = np.ascontiguousarray(np.asarray(proj_w, np.float32).T).astype(BF_NP)
    pb = np.ascontiguousarray(np.asarray(proj_b, np.float32))
    g8 = np.zeros((128, 8), np.float32)
    g8[np.arange(128), np.arange(128) // 16] = 1.0
    gt8 = np.ascontiguousarray(g8.T)
    shared = dict(wq=wq, wk=wk, wv=wv, pw=pw, bq=bq, bk=bk, bv=bv, pb=pb,
                  gns=np.ascontiguousarray(np.asarray(norm_scale, np.float32)),
                  gnb=np.ascontiguousarray(np.asarray(norm_bias, np.float32)),
                  g8=g8, gt8=gt8)
    in_maps = []
    for i in range(B):
        m = dict(shared)
        m["x"] = np.ascontiguousarray(x[i].reshape(C, T))
        in_maps.append(m)
    return in_maps


def kernel(x, norm_scale, norm_bias, qkv_w, qkv_b, proj_w, proj_b):
    x = np.asarray(x, dtype=np.float32)
    B, Cc, Hh, Ww = x.shape
    nc = get_nc()
    in_maps = make_in_maps(x, norm_scale, norm_bias, qkv_w, qkv_b, proj_w, proj_b)
    res = run_bass_kernel_spmd(nc, in_maps, core_ids=list(range(B)))
    out = np.stack([res.results[i]["out"] for i in range(B)])
    return out.reshape(B, Cc, Hh, Ww).astype(np.float32)


# revision 9
# speedup vs baseline: 1.3183x; 1.3183x over previous
"""AttentionBlock (GroupNorm32 + QKV 8-head attention + proj + residual) on 8 TRN2 NeuronCores.

Sharding: pure data-parallel over batch B=8 — one batch element per core.
Per core: x [512, 1024] f32 -> GroupNorm(32) -> qkv (bf16 matmuls) ->
8-head attention (softmax without max-subtraction; logits O(1)) -> proj + residual.

Layout tricks:
  - All big matmuls in bf16 (residual path stays exact f32, so output error ~1e-3).
  - Attention weights computed TRANSPOSED: wT[s, t] = k.T @ q, so the AV matmul needs
    no transposes; V is produced transposed by swapping the qkv matmul operands.
  - Softmax row-sums come free via a ones-column appended to the stationary vT operand.
  - Softmax division is a per-t (free axis) scale: reciprocal row -> PE broadcast
    matmul (ones lhsT) -> fused into the PSUM evacuation multiply.
"""

import numpy as np
import ml_dtypes
from contextlib import ExitStack

import concourse.bass as bass
import concourse.tile as tile
from concourse import bacc, mybir
from concourse.bass_utils import run_bass_kernel_spmd

F32 = mybir.dt.float32
BF = mybir.dt.bfloat16
MULT = mybir.AluOpType.mult
ADD = mybir.AluOpType.add
SUB = mybir.AluOpType.subtract
AFT = mybir.ActivationFunctionType

C, T, H, CH = 512, 1024, 8, 64
NJ = C // 128          # 4 c-tiles
NTM = T // 128         # 8 t-tiles
EPS = 1e-5
EXP_SCALE = float(CH) ** -0.5  # folded (q*s)·(k*s) scale, s = ch**-0.25

BF_NP = ml_dtypes.bfloat16


def build_graph(enable_asserts: bool = False):
    nc = bacc.Bacc(
        "TRN2",
        target_bir_lowering=False,
        debug=False,
        enable_asserts=enable_asserts,
    )
    x_d = nc.dram_tensor("x", [C, T], F32, kind="ExternalInput").ap()
    wq_d = nc.dram_tensor("wq", [C, C], BF, kind="ExternalInput").ap()
    wk_d = nc.dram_tensor("wk", [C, C], BF, kind="ExternalInput").ap()
    wv_d = nc.dram_tensor("wv", [C, C], BF, kind="ExternalInput").ap()
    pw_d = nc.dram_tensor("pw", [C, C], BF, kind="ExternalInput").ap()
    bq_d = nc.dram_tensor("bq", [C], F32, kind="ExternalInput").ap()
    bk_d = nc.dram_tensor("bk", [C], F32, kind="ExternalInput").ap()
    bv_d = nc.dram_tensor("bv", [C], F32, kind="ExternalInput").ap()
    pb_d = nc.dram_tensor("pb", [C], F32, kind="ExternalInput").ap()
    gns_d = nc.dram_tensor("gns", [C], F32, kind="ExternalInput").ap()
    gnb_d = nc.dram_tensor("gnb", [C], F32, kind="ExternalInput").ap()
    g8_d = nc.dram_tensor("g8", [128, 8], F32, kind="ExternalInput").ap()
    gt8_d = nc.dram_tensor("gt8", [8, 128], F32, kind="ExternalInput").ap()
    sel8_d = nc.dram_tensor("sel8", [8, 4 * 128], BF, kind="ExternalInput").ap()
    out_d = nc.dram_tensor("out", [C, T], F32, kind="ExternalOutput").ap()

    with tile.TileContext(nc) as tc, ExitStack() as ctx:
        consts = ctx.enter_context(tc.tile_pool(name="consts", bufs=1))
        bigs = ctx.enter_context(tc.tile_pool(name="bigs", bufs=1))
        ewp = ctx.enter_context(tc.tile_pool(name="ewp", bufs=2))
        work = ctx.enter_context(tc.tile_pool(name="work", bufs=3))
        outp = ctx.enter_context(tc.tile_pool(name="outp", bufs=2))
        qk_ps = ctx.enter_context(tc.tile_pool(name="qk_ps", bufs=2, space="PSUM"))
        av_ps = ctx.enter_context(tc.tile_pool(name="av_ps", bufs=3, space="PSUM"))

        # ---- persistent sbuf tensors ----
        xt = bigs.tile([128, NJ, T], F32)       # raw x, kept for residual
        xn = bigs.tile([128, NJ, T], BF)        # groupnormed x
        q_sb = bigs.tile([128, NJ, T], BF)      # q rows (head-major)
        k_sb = bigs.tile([128, NJ, T], BF)      # k rows (head-major)
        vT_sb = bigs.tile([128, NTM, H, CH + 1], BF)  # v transposed + ones col
        a_sb = bigs.tile([128, NJ, T], BF)      # normalized attention output

        # ---- input DMAs (ordered by first use) ----
        for j in range(NJ):
            nc.sync.dma_start(xt[:, j, :], x_d[j * 128:(j + 1) * 128, :])
        gns_sb = consts.tile([128, NJ], F32)
        gnb_sb = consts.tile([128, NJ], F32)
        nc.sync.dma_start(gns_sb[:], bass.AP(tensor=gns_d.tensor, offset=0, ap=[[1, 128], [128, NJ]]))
        nc.sync.dma_start(gnb_sb[:], bass.AP(tensor=gnb_d.tensor, offset=0, ap=[[1, 128], [128, NJ]]))
        g8_sb = consts.tile([128, 8], F32)
        gt8_sb = consts.tile([8, 128], F32)
        nc.sync.dma_start(g8_sb[:], g8_d[:])
        nc.sync.dma_start(gt8_sb[:], gt8_d[:])

        wq_sb = consts.tile([128, NJ, C], BF)
        wk_sb = consts.tile([128, NJ, C], BF)
        wv_sb = consts.tile([128, NJ, C], BF)
        pw_sb = consts.tile([128, NJ, C], BF)
        for j in range(NJ):
            nc.sync.dma_start(wk_sb[:, j, :], wk_d[j * 128:(j + 1) * 128, :])
            nc.sync.dma_start(wq_sb[:, j, :], wq_d[j * 128:(j + 1) * 128, :])
            nc.sync.dma_start(wv_sb[:, j, :], wv_d[j * 128:(j + 1) * 128, :])
            nc.sync.dma_start(pw_sb[:, j, :], pw_d[j * 128:(j + 1) * 128, :])
        bq_sb = consts.tile([128, NJ], F32)
        bk_sb = consts.tile([128, NJ], F32)
        pb_sb = consts.tile([128, NJ], F32)
        for j in range(NJ):
            nc.sync.dma_start(bq_sb[:, j:j + 1], bq_d[j * 128:(j + 1) * 128])
            nc.sync.dma_start(bk_sb[:, j:j + 1], bk_d[j * 128:(j + 1) * 128])
            nc.sync.dma_start(pb_sb[:, j:j + 1], pb_d[j * 128:(j + 1) * 128])
        bv_bc = consts.tile([128, C], F32)      # v bias broadcast to all partitions
        nc.sync.dma_start(bv_bc[:], bass.AP(tensor=bv_d.tensor, offset=0, ap=[[0, 128], [1, C]]))
        sel8_sb = consts.tile([8, 4, 128], BF)
        nc.sync.dma_start(sel8_sb[:], sel8_d[:].rearrange("p (j m) -> p j m", j=4))
        eps_sb = consts.tile([128, 1], F32)
        nc.vector.memset(eps_sb[:], EPS)
        zero_sb = consts.tile([128, 1], F32)
        nc.vector.memset(zero_sb[:], 0.0)

        # ---- GroupNorm: per-partition stats, group-reduce via tiny f32 matmuls ----
        stats_sb = consts.tile([128, 3 * NJ], F32)  # mean | var | mean^2 per c-tile
        for j in range(NJ):
            st6 = work.tile([128, 2, 6], F32, tag="st6")
            nc.vector.bn_stats(st6[:, 0, :], xt[:, j, 0:512])
            nc.vector.bn_stats(st6[:, 1, :], xt[:, j, 512:1024])
            nc.vector.bn_aggr(stats_sb[:, 3 * j:3 * j + 2], st6[:])
            nc.vector.tensor_mul(stats_sb[:, 3 * j + 2:3 * j + 3],
                                 stats_sb[:, 3 * j:3 * j + 1],
                                 stats_sb[:, 3 * j:3 * j + 1])
        ps_st = av_ps.tile([8, 3 * NJ], F32, tag="av")
        nc.tensor.matmul(ps_st[:], g8_sb[:], stats_sb[:], start=True, stop=True)
        st_g = work.tile([8, 3 * NJ], F32, tag="stg")
        nc.vector.tensor_scalar(st_g[:], ps_st[:], 1.0 / 16.0, None, op0=MULT)
        stv = st_g[:].rearrange("p (j c) -> p j c", c=3)
        bcin = work.tile([8, 8], F32, tag="bcin")
        vv = work.tile([8, NJ], F32, tag="vv")
        nc.vector.tensor_add(vv[:], stv[:, :, 1], stv[:, :, 2])
        m2 = work.tile([8, NJ], F32, tag="m2")
        nc.vector.tensor_mul(m2[:], stv[:, :, 0], stv[:, :, 0])
        nc.vector.tensor_sub(vv[:], vv[:], m2[:])
        nc.scalar.activation(vv[:], vv[:], AFT.Sqrt, bias=eps_sb[0:8, :], scale=1.0)
        nc.vector.tensor_copy(bcin[:, 0:4], stv[:, :, 0])
        nc.vector.reciprocal(bcin[:, 4:8], vv[:])
        ps_pp = av_ps.tile([128, 8], F32, tag="av")
        nc.tensor.matmul(ps_pp[:], gt8_sb[:], bcin[:], start=True, stop=True)
        ab = work.tile([128, 2 * NJ], F32, tag="ab")   # scale | shift per c-tile
        t1 = work.tile([128, 1], F32, tag="t1")
        for j in range(NJ):
            nc.vector.tensor_mul(ab[:, j:j + 1], ps_pp[:, 4 + j:5 + j], gns_sb[:, j:j + 1])
            nc.vector.tensor_mul(t1[:], ps_pp[:, j:j + 1], ab[:, j:j + 1])
            nc.vector.tensor_sub(ab[:, 4 + j:5 + j], gnb_sb[:, j:j + 1], t1[:])
        for j in range(NJ):
            nc.vector.tensor_scalar(xn[:, j, :], xt[:, j, :],
                                    ab[:, j:j + 1], ab[:, 4 + j:5 + j],
                                    op0=MULT, op1=ADD)

        # ---- QKV ----
        # k, q: out[o, t] = Wk/Wq^T(lhsT [c,o]) x xn[c, t]
        for m in range(NJ):
            psk = qk_ps.tile([128, T], F32, tag="qk")
            for n in range(2):
                for j in range(NJ):
                    nc.tensor.matmul(psk[:, 512 * n:512 * (n + 1)],
                                     wk_sb[:, j, 128 * m:128 * (m + 1)],
                                     xn[:, j, 512 * n:512 * (n + 1)],
                                     start=(j == 0), stop=(j == NJ - 1))
            nc.vector.tensor_scalar(k_sb[:, m, :], psk[:], bk_sb[:, m:m + 1], None, op0=ADD)
            psq = qk_ps.tile([128, T], F32, tag="qk")
            for n in range(2):
                for j in range(NJ):
                    nc.tensor.matmul(psq[:, 512 * n:512 * (n + 1)],
                                     wq_sb[:, j, 128 * m:128 * (m + 1)],
                                     xn[:, j, 512 * n:512 * (n + 1)],
                                     start=(j == 0), stop=(j == NJ - 1))
            nc.vector.tensor_scalar(q_sb[:, m, :], psq[:], bq_sb[:, m:m + 1], None, op0=ADD)
        # vT: out[t, ov] = xn(lhsT [c,t]) x Wv([c, ov])
        for tm in range(NTM):
            psv = qk_ps.tile([128, T], F32, tag="qk")
            for j in range(NJ):
                nc.tensor.matmul(psv[:, 0:512],
                                 xn[:, j, 128 * tm:128 * (tm + 1)],
                                 wv_sb[:, j, :],
                                 start=(j == 0), stop=(j == NJ - 1))
            nc.vector.tensor_add(vT_sb[:, tm, :, 0:CH],
                                 psv[:, 0:512].rearrange("p (h c) -> p h c", h=H),
                                 bv_bc[:].rearrange("p (h c) -> p h c", h=H))
            nc.vector.memset(vT_sb[:, tm, :, CH:CH + 1], 1.0)

        # ---- attention, head pairs (2p at partitions 0:64, 2p+1 at 64:128) ----
        # QK of pair p+1 is issued before AV of pair p so the PE queue never
        # stalls behind the ScalarE exp drain of the current pair.
        def emit_qk(p):
            ew = ewp.tile([128, NTM, 2, T], BF, tag="ew")
            for sm in range(NTM):
                for n in range(2):
                    psw = qk_ps.tile([128, T], F32, tag="qk")
                    nc.tensor.matmul(psw[:, 0:512],
                                     k_sb[0:64, p, 128 * sm:128 * (sm + 1)],
                                     q_sb[0:64, p, 512 * n:512 * (n + 1)],
                                     start=True, stop=True, tile_position=(0, 0))
                    nc.tensor.matmul(psw[:, 512:1024],
                                     k_sb[64:128, p, 128 * sm:128 * (sm + 1)],
                                     q_sb[64:128, p, 512 * n:512 * (n + 1)],
                                     start=True, stop=True, tile_position=(64, 0))
                    nc.scalar.activation(ew[:, sm, :, 512 * n:512 * (n + 1)],
                                         psw[:].rearrange("p (u t) -> p u t", u=2),
                                         AFT.Exp, bias=zero_sb[:], scale=EXP_SCALE)
            return ew

        # unnormalized a goes straight into a_sb; row-sums are staged on
        # partition 64 (DVE outputs must start at partition 0/32/64/96), then
        # DMA-scattered to rs8 partitions 2p..2p+1 for one batched reciprocal.
        rs8 = consts.tile([8, 2, 512], F32)

        def emit_av(p, ew):
            rs_row = work.tile([65, 2, 2, 512], F32, tag="rsrow")
            for u in range(2):
                h = 2 * p + u
                for n in range(2):
                    psa = av_ps.tile([CH + 1, 512], F32, tag="av")
                    for sm in range(NTM):
                        nc.tensor.matmul(psa[:],
                                         vT_sb[:, sm, h, :],
                                         ew[:, sm, u, 512 * n:512 * (n + 1)],
                                         start=(sm == 0), stop=(sm == NTM - 1))
                    nc.vector.tensor_copy(a_sb[64 * u:64 * (u + 1), p, 512 * n:512 * (n + 1)],
                                          psa[0:CH, :])
                    nc.vector.tensor_copy(rs_row[64:65, u, n, :], psa[CH:CH + 1, :])
            nc.sync.dma_start(rs8[2 * p:2 * p + 2, :, :], rs_row[64:65, :, :, :])

        ews = {}
        ews[0] = emit_qk(0)
        ews[1] = emit_qk(1)
        emit_av(0, ews.pop(0))
        ews[2] = emit_qk(2)
        emit_av(1, ews.pop(1))
        ews[3] = emit_qk(3)
        emit_av(2, ews.pop(2))
        emit_av(3, ews.pop(3))

        # ---- batched softmax normalization ----
        rc8 = work.tile([8, 2, 512], F32, tag="rc8")
        nc.vector.reciprocal(rc8[:], rs8[:])
        rcb = work.tile([8, 2, 512], BF, tag="rcb")
        nc.vector.tensor_copy(rcb[:], rc8[:])
        for p in range(NJ):
            for n in range(2):
                psb = av_ps.tile([128, 512], F32, tag="av")
                nc.tensor.matmul(psb[:], sel8_sb[:, p, :], rcb[:, n, :],
                                 start=True, stop=True)
                nc.vector.tensor_mul(a_sb[:, p, 512 * n:512 * (n + 1)],
                                     a_sb[:, p, 512 * n:512 * (n + 1)], psb[:])

        # ---- proj + residual ----
        for m in range(NJ):
            psp = qk_ps.tile([128, T], F32, tag="qk")
            for n in range(2):
                for j in range(NJ):
                    nc.tensor.matmul(psp[:, 512 * n:512 * (n + 1)],
                                     pw_sb[:, j, 128 * m:128 * (m + 1)],
                                     a_sb[:, j, 512 * n:512 * (n + 1)],
                                     start=(j == 0), stop=(j == NJ - 1))
            osb = outp.tile([128, T], F32, tag="osb")
            nc.vector.scalar_tensor_tensor(osb[:], psp[:], pb_sb[:, m:m + 1], xt[:, m, :],
                                           op0=ADD, op1=ADD)
            nc.sync.dma_start(out_d[128 * m:128 * (m + 1), :], osb[:])

    nc.compile()
    return nc


_NC_CACHE = {}


def get_nc():
    if "nc" not in _NC_CACHE:
        _NC_CACHE["nc"] = build_graph()
    return _NC_CACHE["nc"]


def make_in_maps(x, norm_scale, norm_bias, qkv_w, qkv_b, proj_w, proj_b):
    x = np.asarray(x, dtype=np.float32)
    B = x.shape[0]
    qr = np.asarray(qkv_w, np.float32).reshape(H, 3, CH, C)
    wq = np.ascontiguousarray(qr[:, 0].reshape(C, C).T).astype(BF_NP)
    wk = np.ascontiguousarray(qr[:, 1].reshape(C, C).T).astype(BF_NP)
    wv = np.ascontiguousarray(qr[:, 2].reshape(C, C).T).astype(BF_NP)
    br = np.asarray(qkv_b, np.float32).reshape(H, 3, CH)
    bq = np.ascontiguousarray(br[:, 0].reshape(C))
    bk = np.ascontiguousarray(br[:, 1].reshape(C))
    bv = np.ascontiguousarray(br[:, 2].reshape(C))
    pw = np.ascontiguousarray(np.asarray(proj_w, np.float32).T).astype(BF_NP)
    pb = np.ascontiguousarray(np.asarray(proj_b, np.float32))
    g8 = np.zeros((128, 8), np.float32)
    g8[np.arange(128), np.arange(128) // 16] = 1.0
    gt8 = np.ascontiguousarray(g8.T)
    sel8 = np.zeros((8, 4, 128), np.float32)
    for p_ in range(4):
        sel8[2 * p_, p_, 0:64] = 1.0
        sel8[2 * p_ + 1, p_, 64:128] = 1.0
    sel8 = np.ascontiguousarray(sel8.reshape(8, 512)).astype(BF_NP)
    shared = dict(wq=wq, wk=wk, wv=wv, pw=pw, bq=bq, bk=bk, bv=bv, pb=pb,
                  sel8=sel8,
                  gns=np.ascontiguousarray(np.asarray(norm_scale, np.float32)),
                  gnb=np.ascontiguousarray(np.asarray(norm_bias, np.float32)),
                  g8=g8, gt8=gt8)
    in_maps = []
    for i in range(B):
        m = dict(shared)
        m["x"] = np.ascontiguousarray(x[i].reshape(C, T))
        in_maps.append(m)
    return in_maps


def kernel(x, norm_scale, norm_bias, qkv_w, qkv_b, proj_w, proj_b):
    x = np.asarray(x, dtype=np.float32)
    B, Cc, Hh, Ww = x.shape
    nc = get_nc()
    in_maps = make_in_maps(x, norm_scale, norm_bias, qkv_w, qkv_b, proj_w, proj_b)
    res = run_bass_kernel_spmd(nc, in_maps, core_ids=list(range(B)))
    out = np.stack([res.results[i]["out"] for i in range(B)])
    return out.reshape(B, Cc, Hh, Ww).astype(np.float32)
